# revision 8
# baseline (speedup 1.0000x reference)
"""Trainium2 Bass kernel for NeuralRecurrentDynamicsModel.

Data-parallel over batch across 8 NeuronCores (256 rows/core).
Feature-major activations [features, batch]; fp32r matmuls (N=256 moving);
Mamba selective scan via DVE tensor_tensor_scan with d-on-partitions layout.
"""
import sys
import numpy as np

if "/opt/trn_rl_repo" not in sys.path:
    sys.path.insert(0, "/opt/trn_rl_repo")

import concourse.bass as bass
import concourse.bacc as bacc
import concourse.mybir as mybir
import concourse.tile as tile
from concourse.bass_utils import run_bass_kernel_spmd

F32 = mybir.dt.float32
F32R = mybir.dt.float32r
AF = mybir.ActivationFunctionType
ALU = mybir.AluOpType

B = 256          # batch rows per core
NCORES = 8
OBS = 2048
HID = 4096
PRE = 8192
SEQ = 8
DM = 512
DI = 1024
DS = 16
DTR = 32
CK = 4           # conv width
HT = HID // 128
PT = PRE // 128
MT1 = OBS // 128
DIT = DI // 128
DMT = DM // 128
NPG = 2          # n per scan group
NG = DS // NPG   # 8 groups
BH = 2
TOK = SEQ * 128  # tokens per batch-half, l-major (free = l*128 + b)

_BUILD_CACHE = {}


def dap(t, off, dims):
    a = t[...]
    return bass.AP(tensor=a.tensor, offset=a.offset + off, ap=dims)


def build_kernel(an_scales):
    nc = bacc.Bacc()

    xcatT = nc.dram_tensor("xcatT", [PRE, B], F32R, kind="ExternalInput")
    w0t = nc.dram_tensor("w0t", [HT, PT, 128, 128], F32R, kind="ExternalInput")
    w1t = nc.dram_tensor("w1t", [HT, HT, 128, 128], F32R, kind="ExternalInput")
    w2t = nc.dram_tensor("w2t", [HT, HT, 128, 128], F32R, kind="ExternalInput")
    w3t = nc.dram_tensor("w3t", [HT, HT, 128, 128], F32R, kind="ExternalInput")
    mlpb = nc.dram_tensor("mlpb", [4, HT, 128], F32, kind="ExternalInput")
    wih = nc.dram_tensor("wih", [3, HT, HT, 128, 128], F32R, kind="ExternalInput")
    whh = nc.dram_tensor("whh", [3, HT, HT, 128, 128], F32R, kind="ExternalInput")
    gbias = nc.dram_tensor("gbias", [4, HT, 128], F32, kind="ExternalInput")
    mb = []
    for i in range(3):
        mb.append(dict(
            inproj=nc.dram_tensor(f"inproj{i}", [2 * DIT, DMT, 128, 128], F32R, kind="ExternalInput"),
            convw=nc.dram_tensor(f"convw{i}", [DIT, 128, CK], F32, kind="ExternalInput"),
            convb=nc.dram_tensor(f"convb{i}", [DIT, 128], F32, kind="ExternalInput"),
            xproj=nc.dram_tensor(f"xproj{i}", [DIT, 128, 64], F32R, kind="ExternalInput"),
            dtw=nc.dram_tensor(f"dtw{i}", [DIT, DTR, 128], F32R, kind="ExternalInput"),
            dtb=nc.dram_tensor(f"dtb{i}", [DIT, 128], F32, kind="ExternalInput"),
            Dp=nc.dram_tensor(f"Dp{i}", [DIT, 128], F32, kind="ExternalInput"),
            outproj=nc.dram_tensor(f"outproj{i}", [DIT, DMT, 128, 128], F32R, kind="ExternalInput"),
        ))
    mlp1t = nc.dram_tensor("mlp1t", [MT1, HT, 128, 128], F32R, kind="ExternalInput")
    mlp1b = nc.dram_tensor("mlp1b", [MT1, 128], F32, kind="ExternalInput")

    obsT = nc.dram_tensor("obsT", [MT1, 128, B], F32, kind="ExternalOutput")
    hhatT = nc.dram_tensor("hhatT", [HT, 128, B], F32, kind="ExternalOutput")

    ybuf = nc.dram_tensor("ybuf", [HT, 128, B], F32R)
    xblk = [nc.dram_tensor(f"xblk{i}", [DMT, 128, SEQ, B], F32R) for i in range(3)]
    zbuf = [nc.dram_tensor(f"zbuf{i}", [DIT, 128, SEQ, B], F32) for i in range(3)]
    dtbuf = [nc.dram_tensor(f"dtbuf{i}", [DIT, 128, SEQ, B], F32) for i in range(3)]
    ubuf = [nc.dram_tensor(f"ubuf{i}", [DIT, 128, SEQ, B], F32) for i in range(3)]
    xcbuf = [nc.dram_tensor(f"xcbuf{i}", [DIT, 128, SEQ, B], F32R) for i in range(3)]
    ygbuf = [nc.dram_tensor(f"ygbuf{i}", [DIT, 128, SEQ, B], F32R) for i in range(3)]
    yscan = [nc.dram_tensor(f"yscan{i}", [DIT, 128, B, SEQ], F32) for i in range(3)]
    bcbuf = [nc.dram_tensor(f"bcbuf{i}", [2 * DS, TOK], F32) for i in range(3)]

    with tile.TileContext(nc) as tc:
        # =================== MLP0 ===================
        with tc.tile_pool(name="mlp0w", bufs=4) as wpool, \
             tc.tile_pool(name="mlp0misc", bufs=1) as mpool, \
             tc.tile_pool(name="mlp0ps", bufs=4, space="PSUM") as pspool:
            biases = mpool.tile([128, 4, HT], F32, name="biases")
            nc.sync.dma_start(out=biases, in_=mlpb.rearrange("g t p -> p g t"))
            y_prev = None
            for li, wt_d in enumerate((w0t, w1t, w2t, w3t)):
                kt_n = PT if li == 0 else HT
                y_cur = mpool.tile([128, HT, B], F32R, tag=f"y{li % 2}", name=f"y{li}")
                for mt in range(HT):
                    ps = pspool.tile([128, B], F32, tag="ps", name="ps0")
                    for kt in range(kt_n):
                        wt_s = wpool.tile([128, 128], F32R, tag="w", name="w0s")
                        nc.sync.dma_start(out=wt_s, in_=wt_d[mt, kt, :, :])
                        if li == 0:
                            rhs_t = wpool.tile([128, B], F32R, tag="r", name="r0s")
                            nc.sync.dma_start(out=rhs_t, in_=xcatT[kt * 128:(kt + 1) * 128, :])
                            rhs = rhs_t[:]
                        else:
                            rhs = y_prev[:, kt, :]
                        nc.tensor.matmul(ps[:], wt_s[:], rhs, start=(kt == 0), stop=(kt == kt_n - 1))
                    func = AF.Relu if li < 3 else AF.Identity
                    nc.scalar.activation(y_cur[:, mt, :], ps[:], func, bias=biases[:, li, mt:mt + 1])
                y_prev = y_cur
            for mt in range(HT):
                nc.sync.dma_start(out=ybuf[mt, :, :], in_=y_prev[:, mt, :])

        # =================== MAMBA + GRU ===================
        with tc.tile_pool(name="msb", bufs=1) as sbp, \
             tc.tile_pool(name="mw", bufs=4) as wstr, \
             tc.tile_pool(name="mfm", bufs=1) as fm_p, \
             tc.tile_pool(name="mscan", bufs=1) as scan_p, \
             tc.tile_pool(name="mtmp", bufs=2) as tmp_p, \
             tc.tile_pool(name="mps", bufs=2, space="PSUM") as ps_m:

            def mamba_block(i):
                m = mb[i]
                an = an_scales[i]
                cvec = sbp.tile([128, DIT, CK], F32, tag="convw", name="cvec")
                nc.sync.dma_start(out=cvec, in_=m["convw"].rearrange("t p k -> p t k"))
                cb = sbp.tile([128, DIT], F32, tag="convb", name="cb")
                nc.sync.dma_start(out=cb, in_=m["convb"].rearrange("t p -> p t"))
                dtb_t = sbp.tile([128, DIT], F32, tag="dtb", name="dtb_t")
                nc.sync.dma_start(out=dtb_t, in_=m["dtb"].rearrange("t p -> p t"))
                dp_t = sbp.tile([128, DIT], F32, tag="Dp", name="dp_t")
                nc.sync.dma_start(out=dp_t, in_=m["Dp"].rearrange("t p -> p t"))
                for bh in range(BH):
                    b0 = bh * 128
                    tok_dims = [[SEQ * B, 128], [B, SEQ], [1, 128]]

                    def tok_ap(buf, dt_i):
                        return dap(buf, dt_i * 128 * SEQ * B + b0, tok_dims)

                    # ---- in_proj ----
                    rhs_tiles = []
                    for dmt in range(DMT):
                        rt = tmp_p.tile([128, SEQ, 128], F32R, tag="inrhs", bufs=4, name="inrhs")
                        if i == 0:
                            nc.sync.dma_start(out=rt, in_=dap(
                                ybuf, dmt * 128 * B + b0,
                                [[B, 128], [4 * 128 * B, SEQ], [1, 128]]))
                        else:
                            nc.sync.dma_start(out=rt, in_=tok_ap(xblk[i - 1], dmt))
                        rhs_tiles.append(rt)
                    for mt in range(2 * DIT):
                        ps = ps_m.tile([128, SEQ * 128], F32, tag="big", name="ipps")
                        for kt in range(DMT):
                            wt_s = wstr.tile([128, 128], F32R, tag="ipw", name="ipw")
                            nc.sync.dma_start(out=wt_s, in_=m["inproj"][mt, kt, :, :])
                            rf = rhs_tiles[kt][:, :, :].rearrange("p a b -> p (a b)")
                            for h_ in range(2):
                                nc.tensor.matmul(ps[:, h_ * 512:(h_ + 1) * 512], wt_s[:],
                                                 rf[:, h_ * 512:(h_ + 1) * 512],
                                                 start=(kt == 0), stop=(kt == DMT - 1))
                        if mt < DIT:
                            dt_i = mt
                            xp = fm_p.tile([128, 3 + SEQ, 128], F32, tag="xipad", bufs=2, name="xp")
                            nc.vector.memset(xp[:, 0:3, :], 0.0)
                            nc.scalar.activation(
                                xp[:, 3:3 + SEQ, :].rearrange("p a b -> p (a b)"), ps[:], AF.Copy)
                            c0 = fm_p.tile([128, SEQ * 128], F32, tag="cacc0", name="c0")
                            c1 = fm_p.tile([128, SEQ * 128], F32, tag="cacc1", name="c1")
                            sh = lambda j: xp[:, j:j + SEQ, :].rearrange("p a b -> p (a b)")
                            nc.vector.tensor_scalar_mul(c0[:], sh(0), cvec[:, dt_i, 0:1])
                            nc.vector.scalar_tensor_tensor(c1[:], sh(1), cvec[:, dt_i, 1:2], c0[:], ALU.mult, ALU.add)
                            nc.vector.scalar_tensor_tensor(c0[:], sh(2), cvec[:, dt_i, 2:3], c1[:], ALU.mult, ALU.add)
                            nc.vector.scalar_tensor_tensor(c1[:], sh(3), cvec[:, dt_i, 3:4], c0[:], ALU.mult, ALU.add)
                            xct = fm_p.tile([128, SEQ * 128], F32R, tag="xct", bufs=2, name="xct")
                            nc.scalar.activation(xct[:], c1[:], AF.Silu, bias=cb[:, dt_i:dt_i + 1])
                            nc.sync.dma_start(out=tok_ap(xcbuf[i], dt_i), in_=xct[:])
                        else:
                            zev = fm_p.tile([128, SEQ * 128], F32, tag="zev", bufs=1, name="zev")
                            nc.scalar.activation(zev[:], ps[:], AF.Copy)
                            nc.sync.dma_start(out=tok_ap(zbuf[i], mt - DIT), in_=zev[:])
                    # ---- x_proj ----
                    xdbl_ps = ps_m.tile([64, SEQ * 128], F32, tag="big", name="xpps")
                    for kt in range(DIT):
                        wt_s = wstr.tile([128, 64], F32R, tag="xpw", name="xpw")
                        nc.sync.dma_start(out=wt_s, in_=m["xproj"][kt, :, :])
                        xc_kt = fm_p.tile([128, SEQ * 128], F32R, tag="xcs", bufs=2, name="xcs")
                        nc.sync.dma_start(out=xc_kt, in_=tok_ap(xcbuf[i], kt))
                        for h_ in range(2):
                            nc.tensor.matmul(xdbl_ps[:, h_ * 512:(h_ + 1) * 512], wt_s[:],
                                             xc_kt[:, h_ * 512:(h_ + 1) * 512],
                                             start=(kt == 0), stop=(kt == DIT - 1))
                    xdbl = sbp.tile([64, SEQ * 128], F32R, tag="xdbl", name="xdbl")
                    nc.scalar.activation(xdbl[:], xdbl_ps[:], AF.Copy)
                    # B,C rows -> DRAM for partition-broadcast
                    nc.sync.dma_start(out=bcbuf[i][:, :], in_=xdbl[DTR:DTR + 2 * DS, :].bitcast(F32))
                    # ---- dt, u ----
                    for dt_i in range(DIT):
                        wt_s = wstr.tile([DTR, 128], F32R, tag="dtw", name="dtw")
                        nc.sync.dma_start(out=wt_s, in_=m["dtw"][dt_i, :, :])
                        ps = ps_m.tile([128, SEQ * 128], F32, tag="big", name="dtps")
                        for h_ in range(2):
                            nc.tensor.matmul(ps[:, h_ * 512:(h_ + 1) * 512], wt_s[:],
                                             xdbl[0:DTR, h_ * 512:(h_ + 1) * 512],
                                             start=True, stop=True)
                        dtt = fm_p.tile([128, SEQ * 128], F32, tag="dtt", bufs=2, name="dtt")
                        nc.scalar.activation(dtt[:], ps[:], AF.Exp, bias=dtb_t[:, dt_i:dt_i + 1])
                        nc.scalar.activation(dtt[:], dtt[:], AF.Ln, bias=1.0)
                        nc.sync.dma_start(out=tok_ap(dtbuf[i], dt_i), in_=dtt[:])
                        ut = fm_p.tile([128, SEQ * 128], F32, tag="ut", bufs=2, name="ut")
                        xc_kt = fm_p.tile([128, SEQ * 128], F32R, tag="xcs", bufs=2, name="xcs2")
                        nc.sync.dma_start(out=xc_kt, in_=tok_ap(xcbuf[i], dt_i))
                        nc.vector.tensor_tensor(out=ut[:], in0=dtt[:], in1=xc_kt[:].bitcast(F32), op=ALU.mult)
                        nc.sync.dma_start(out=tok_ap(ubuf[i], dt_i), in_=ut[:])
                    # ---- scan ----
                    for ng in range(NG):
                        Bbc = scan_p.tile([128, NPG, SEQ, 128], F32, tag="Bbc", name="Bbc")
                        Cbc = scan_p.tile([128, NPG, SEQ, 128], F32, tag="Cbc", name="Cbc")
                        for (dst, base) in ((Bbc, 0), (Cbc, DS)):
                            for nr in range(NPG):
                                n_ = ng * NPG + nr
                                nc.sync.dma_start(
                                    out=dst[:, nr, :, :],
                                    in_=dap(bcbuf[i], (base + n_) * TOK,
                                            [[0, 128], [1, TOK]]))
                        for dt_i in range(DIT):
                            dtt = fm_p.tile([128, SEQ * 128], F32, tag="dtt", bufs=2, name="dtt2")
                            nc.sync.dma_start(out=dtt, in_=tok_ap(dtbuf[i], dt_i))
                            ut = fm_p.tile([128, SEQ * 128], F32, tag="ut", bufs=2, name="ut2")
                            nc.sync.dma_start(out=ut, in_=tok_ap(ubuf[i], dt_i))
                            dA = scan_p.tile([128, NPG, 128, SEQ], F32, tag="dA", name="dA")
                            dBu = scan_p.tile([128, NPG, 128, SEQ], F32, tag="dBu", name="dBu")
                            hh = scan_p.tile([128, NPG, 128, SEQ], F32, tag="hh", name="hh")
                            dt_bl = dtt[:].rearrange("p (l b) -> p b l", l=SEQ)
                            for nr in range(NPG):
                                n_ = ng * NPG + nr
                                nc.scalar.activation(dA[:, nr, :, :], dt_bl, AF.Exp,
                                                     scale=float(an[n_]))
                            nc.vector.memset(dA[:, :, :, 0:1], 0.0)
                            u_r = ut[:].rearrange("p (l b) -> p b l", l=SEQ)
                            u_b = bass.AP(tensor=u_r.tensor, offset=u_r.offset,
                                          ap=[u_r.ap[0], [0, NPG], u_r.ap[1], u_r.ap[2]])
                            nc.vector.tensor_tensor(out=dBu[:, :, :, :], in0=u_b, in1=Bbc[:, :, :, :].transpose([0, 1, 3, 2]), op=ALU.mult)
                            flat = lambda t: t[:, :, :, :].rearrange("p a b c -> p (a b c)")
                            nc.vector.tensor_tensor_scan(flat(hh), flat(dA), flat(dBu), 0.0, ALU.mult, ALU.add)
                            nc.vector.tensor_tensor(out=dBu[:, :, :, :], in0=hh[:, :, :, :], in1=Cbc[:, :, :, :].transpose([0, 1, 3, 2]), op=ALU.mult)
                            red = tmp_p.tile([128, 128, SEQ], F32, tag="red", name="red")
                            nc.vector.tensor_reduce(out=red[:, :, :], in_=dBu[:, :, :, :].transpose([0, 2, 3, 1]),
                                                    axis=mybir.AxisListType.X, op=ALU.add)
                            ysc_ap = dap(yscan[i], dt_i * 128 * SEQ * B + b0 * SEQ,
                                         [[SEQ * B, 128], [SEQ, 128], [1, SEQ]])
                            if ng == 0:
                                nc.sync.dma_start(out=ysc_ap, in_=red[:, :, :])
                            else:
                                yacc = tmp_p.tile([128, 128, SEQ], F32, tag="yacc", name="yacc")
                                nc.sync.dma_start(out=yacc, in_=ysc_ap)
                                nc.vector.tensor_tensor(out=red[:, :, :], in0=yacc[:, :, :], in1=red[:, :, :], op=ALU.add)
                                nc.sync.dma_start(out=ysc_ap, in_=red[:, :, :])
                    # ---- gate ----
                    for dt_i in range(DIT):
                        ya = tmp_p.tile([128, 128, SEQ], F32, tag="yacc", name="ya")
                        nc.sync.dma_start(out=ya, in_=dap(
                            yscan[i], dt_i * 128 * SEQ * B + b0 * SEQ,
                            [[SEQ * B, 128], [SEQ, 128], [1, SEQ]]))
                        xct = fm_p.tile([128, SEQ * 128], F32R, tag="xcs", bufs=2, name="xcg")
                        nc.sync.dma_start(out=xct, in_=tok_ap(xcbuf[i], dt_i))
                        zt = tmp_p.tile([128, SEQ * 128], F32, tag="zt", bufs=1, name="zt")
                        nc.sync.dma_start(out=zt, in_=tok_ap(zbuf[i], dt_i))
                        zs = tmp_p.tile([128, SEQ * 128], F32, tag="zs", bufs=1, name="zs")
                        nc.scalar.activation(zs[:], zt[:], AF.Silu)
                        yg = tmp_p.tile([128, SEQ, 128], F32, tag="yg", bufs=1, name="yg")
                        nc.vector.scalar_tensor_tensor(
                            yg[:, :, :], xct[:].rearrange("p (l b) -> p l b", l=SEQ).bitcast(F32),
                            dp_t[:, dt_i:dt_i + 1],
                            ya[:, :, :].transpose([0, 2, 1]), ALU.mult, ALU.add)
                        ygr = tmp_p.tile([128, SEQ * 128], F32R, tag="ygr", bufs=1, name="ygr")
                        nc.vector.tensor_tensor(out=ygr[:].bitcast(F32),
                                                in0=yg[:, :, :].rearrange("p a b -> p (a b)"),
                                                in1=zs[:], op=ALU.mult)
                        nc.sync.dma_start(out=tok_ap(ygbuf[i], dt_i), in_=ygr[:])
                    # ---- out_proj ----
                    for dmt in range(DMT):
                        ps = ps_m.tile([128, SEQ * 128], F32, tag="big", name="opps")
                        for kt in range(DIT):
                            wt_s = wstr.tile([128, 128], F32R, tag="opw", name="opw")
                            nc.sync.dma_start(out=wt_s, in_=m["outproj"][kt, dmt, :, :])
                            ygr = fm_p.tile([128, SEQ * 128], F32R, tag="ygs", bufs=2, name="ygs")
                            nc.sync.dma_start(out=ygr, in_=tok_ap(ygbuf[i], kt))
                            for h_ in range(2):
                                nc.tensor.matmul(ps[:, h_ * 512:(h_ + 1) * 512], wt_s[:],
                                                 ygr[:, h_ * 512:(h_ + 1) * 512],
                                                 start=(kt == 0), stop=(kt == DIT - 1))
                        ot = tmp_p.tile([128, SEQ * 128], F32R, tag="oevac", bufs=1, name="ot")
                        nc.scalar.activation(ot[:], ps[:], AF.Copy)
                        nc.sync.dma_start(out=tok_ap(xblk[i], dmt), in_=ot[:])

            def gru():
                gb = sbp.tile([128, 4, HT], F32, tag="gbias", name="gb")
                nc.sync.dma_start(out=gb, in_=gbias.rearrange("g t p -> p g t"))
                with tc.tile_pool(name="gru_w", bufs=2) as gw, \
                     tc.tile_pool(name="gru_rhs", bufs=2) as gr, \
                     tc.tile_pool(name="gru_t", bufs=1) as gt, \
                     tc.tile_pool(name="gru_ps", bufs=1, space="PSUM") as gps:
                    for mt in range(HT):
                        ps_r = gps.tile([128, B], F32, tag="r", name="psr")
                        ps_z = gps.tile([128, B], F32, tag="z", name="psz")
                        ps_in = gps.tile([128, B], F32, tag="in", name="psin")
                        ps_hn = gps.tile([128, B], F32, tag="hn", name="pshn")
                        for kt in range(HT):
                            y3t = gr.tile([128, B], F32R, tag="y3", name="y3t")
                            nc.sync.dma_start(out=y3t, in_=ybuf[kt, :, :])
                            hTt = gr.tile([128, B], F32R, tag="hT", name="hTt")
                            nc.sync.dma_start(out=hTt, in_=xcatT[OBS + kt * 128:OBS + (kt + 1) * 128, :])
                            first = kt == 0
                            last = kt == HT - 1
                            for g, pst in ((0, ps_r), (1, ps_z), (2, ps_in)):
                                wt_s = gw.tile([128, 128], F32R, tag=f"wi{g}", name="wis")
                                nc.sync.dma_start(out=wt_s, in_=wih[g, mt, kt, :, :])
                                nc.tensor.matmul(pst[:], wt_s[:], y3t[:], start=first,
                                                 stop=(last and g == 2))
                            for g, pst in ((0, ps_r), (1, ps_z), (2, ps_hn)):
                                wt_s = gw.tile([128, 128], F32R, tag=f"wh{g}", name="whs")
                                nc.sync.dma_start(out=wt_s, in_=whh[g, mt, kt, :, :])
                                nc.tensor.matmul(pst[:], wt_s[:], hTt[:], start=(first and g == 2),
                                                 stop=last)
                        r_t = gt.tile([128, B], F32, tag="r", bufs=2, name="rt")
                        nc.scalar.activation(r_t[:], ps_r[:], AF.Sigmoid, bias=gb[:, 0, mt:mt + 1])
                        z_t = gt.tile([128, B], F32, tag="z", bufs=2, name="zt2")
                        nc.scalar.activation(z_t[:], ps_z[:], AF.Sigmoid, bias=gb[:, 1, mt:mt + 1])
                        in_t = gt.tile([128, B], F32, tag="in", bufs=2, name="int")
                        nc.scalar.activation(in_t[:], ps_in[:], AF.Identity, bias=gb[:, 2, mt:mt + 1])
                        hn_t = gt.tile([128, B], F32, tag="hn", bufs=2, name="hnt")
                        nc.scalar.activation(hn_t[:], ps_hn[:], AF.Identity, bias=gb[:, 3, mt:mt + 1])
                        t1 = gt.tile([128, B], F32, tag="t1", bufs=2, name="t1")
                        nc.vector.tensor_tensor(out=t1[:], in0=r_t[:], in1=hn_t[:], op=ALU.mult)
                        nc.vector.tensor_tensor(out=t1[:], in0=t1[:], in1=in_t[:], op=ALU.add)
                        n_t = gt.tile([128, B], F32, tag="n", bufs=2, name="nt")
                        nc.scalar.activation(n_t[:], t1[:], AF.Tanh)
                        hsl = gr.tile([128, B], F32R, tag="hT", name="hsl")
                        nc.sync.dma_start(out=hsl, in_=xcatT[OBS + mt * 128:OBS + (mt + 1) * 128, :])
                        t2 = gt.tile([128, B], F32, tag="t2", bufs=2, name="t2")
                        nc.vector.tensor_tensor(out=t2[:], in0=hsl[:].bitcast(F32), in1=n_t[:], op=ALU.subtract)
                        nc.vector.tensor_tensor(out=t2[:], in0=t2[:], in1=z_t[:], op=ALU.mult)
                        nc.vector.tensor_tensor(out=t2[:], in0=t2[:], in1=n_t[:], op=ALU.add)
                        nc.sync.dma_start(out=hhatT[mt, :, :], in_=t2[:])

            mamba_block(0)
            gru()
            mamba_block(1)
            mamba_block(2)

        # =================== MLP1 ===================
        with tc.tile_pool(name="m1w", bufs=4) as wpool, \
             tc.tile_pool(name="m1o", bufs=4) as opool, \
             tc.tile_pool(name="m1ps", bufs=4, space="PSUM") as pspool:
            b1t = opool.tile([128, MT1], F32, tag="b", bufs=1, name="b1t")
            nc.sync.dma_start(out=b1t, in_=mlp1b.rearrange("t p -> p t"))
            for mt in range(MT1):
                ps = pspool.tile([128, B], F32, tag="ps", name="ps1")
                for kt in range(HT):
                    wt_s = wpool.tile([128, 128], F32R, tag="w", name="w1s")
                    nc.sync.dma_start(out=wt_s, in_=mlp1t[mt, kt, :, :])
                    rhs_t = wpool.tile([128, B], F32R, tag="r", name="r1s")
                    nc.sync.dma_start(out=rhs_t, in_=xblk[2][kt % 4, :, kt // 4, :])
                    nc.tensor.matmul(ps[:], wt_s[:], rhs_t[:], start=(kt == 0), stop=(kt == HT - 1))
                ot = opool.tile([128, B], F32, tag="o", name="o1t")
                nc.scalar.activation(ot[:], ps[:], AF.Identity, bias=b1t[:, mt:mt + 1])
                nc.sync.dma_start(out=obsT[mt, :, :], in_=ot[:])

    nc.compile()
    return nc


def _tiles(WT, ktiles, mtiles):
    return np.ascontiguousarray(
        WT.reshape(ktiles, 128, mtiles, 128).transpose(2, 0, 1, 3))


def kernel(obs_latent, h_state, action, params):
    obs_latent = np.asarray(obs_latent, np.float32)
    h_state = np.asarray(h_state, np.float32)
    action = np.asarray(action, np.float32)

    mlp0 = params["mlp0"]
    gru = params["gru"]
    mambas = params["mamba"]
    W1, b1o = params["mlp1"]

    an_scales = []
    for mp in mambas:
        A = -np.exp(np.asarray(mp["A_log"], np.float32))
        assert np.allclose(A, A[0:1, :], atol=1e-5), "A must be d-independent"
        an_scales.append([float(x) for x in A[0]])

    key = tuple(tuple(a) for a in an_scales)
    if key not in _BUILD_CACHE:
        _BUILD_CACHE[key] = build_kernel(an_scales)
    nc = _BUILD_CACHE[key]

    xcat = np.concatenate([obs_latent, h_state, action], axis=1)

    shared = {}
    for li, (W, b) in enumerate(mlp0):
        W = np.asarray(W, np.float32)
        shared[f"w{li}t"] = _tiles(np.ascontiguousarray(W.T), W.shape[1] // 128, W.shape[0] // 128)
    shared["mlpb"] = np.stack([np.asarray(b, np.float32).reshape(HT, 128) for _, b in mlp0])
    wih_ = np.asarray(gru["w_ih"], np.float32)
    whh_ = np.asarray(gru["w_hh"], np.float32)
    shared["wih"] = np.stack([_tiles(np.ascontiguousarray(wih_[g * HID:(g + 1) * HID, :].T), HT, HT) for g in range(3)])
    shared["whh"] = np.stack([_tiles(np.ascontiguousarray(whh_[g * HID:(g + 1) * HID, :].T), HT, HT) for g in range(3)])
    bih = np.asarray(gru["b_ih"], np.float32)
    bhh = np.asarray(gru["b_hh"], np.float32)
    shared["gbias"] = np.stack([
        (bih[0:HID] + bhh[0:HID]).reshape(HT, 128),
        (bih[HID:2 * HID] + bhh[HID:2 * HID]).reshape(HT, 128),
        bih[2 * HID:].reshape(HT, 128),
        bhh[2 * HID:].reshape(HT, 128)])
    for i, mp in enumerate(mambas):
        ip = np.asarray(mp["in_proj"], np.float32)
        shared[f"inproj{i}"] = _tiles(np.ascontiguousarray(ip.T), DMT, 2 * DIT)
        shared[f"convw{i}"] = np.asarray(mp["conv_w"], np.float32).reshape(DIT, 128, CK)
        shared[f"convb{i}"] = np.asarray(mp["conv_b"], np.float32).reshape(DIT, 128)
        xp_ = np.asarray(mp["x_proj"], np.float32)
        shared[f"xproj{i}"] = np.ascontiguousarray(xp_.T.reshape(DIT, 128, 64))
        dw = np.asarray(mp["dt_w"], np.float32)
        shared[f"dtw{i}"] = np.ascontiguousarray(dw.T.reshape(DTR, DIT, 128).transpose(1, 0, 2))
        shared[f"dtb{i}"] = np.asarray(mp["dt_b"], np.float32).reshape(DIT, 128)
        shared[f"Dp{i}"] = np.asarray(mp["D"], np.float32).reshape(DIT, 128)
        op_ = np.asarray(mp["out_proj"], np.float32)
        shared[f"outproj{i}"] = _tiles(np.ascontiguousarray(op_.T), DIT, DMT)
    W1 = np.asarray(W1, np.float32)
    shared["mlp1t"] = _tiles(np.ascontiguousarray(W1.T), HT, MT1)
    shared["mlp1b"] = np.asarray(b1o, np.float32).reshape(MT1, 128)

    in_maps = []
    for c in range(NCORES):
        im = dict(shared)
        im["xcatT"] = np.ascontiguousarray(xcat[c * B:(c + 1) * B].T)
        in_maps.append(im)

    res = run_bass_kernel_spmd(nc, in_maps, core_ids=list(range(NCORES)))

    obs_out = np.empty((NCORES * B, OBS), np.float32)
    h_out = np.empty((NCORES * B, HID), np.float32)
    for c in range(NCORES):
        obs_out[c * B:(c + 1) * B] = res.results[c]["obsT"].reshape(OBS, B).T
        h_out[c * B:(c + 1) * B] = res.results[c]["hhatT"].reshape(HID, B).T
    return obs_out, h_out


# revision 9
# speedup vs baseline: 12754.7897x; 12754.7897x over previous
"""Trainium2 Bass kernel for NeuralRecurrentDynamicsModel.

Data-parallel over batch across 8 NeuronCores (256 rows/core).
Feature-major activations [features, batch]; fp32r matmuls (N=256 moving);
Mamba selective scan via DVE tensor_tensor_scan with d-on-partitions layout.
"""
import sys
import numpy as np

if "/opt/trn_rl_repo" not in sys.path:
    sys.path.insert(0, "/opt/trn_rl_repo")

import concourse.bass as bass
import concourse.bacc as bacc
import concourse.mybir as mybir
import concourse.tile as tile
from concourse.bass_utils import run_bass_kernel_spmd

F32 = mybir.dt.float32
F32R = mybir.dt.float32r
AF = mybir.ActivationFunctionType
ALU = mybir.AluOpType

B = 256          # batch rows per core
NCORES = 8
OBS = 2048
HID = 4096
PRE = 8192
SEQ = 8
DM = 512
DI = 1024
DS = 16
DTR = 32
CK = 4           # conv width
HT = HID // 128
PT = PRE // 128
MT1 = OBS // 128
DIT = DI // 128
DMT = DM // 128
NPG = 2          # n per scan group
NG = DS // NPG   # 8 groups
BH = 2
TOK = SEQ * 128  # tokens per batch-half, l-major (free = l*128 + b)

_BUILD_CACHE = {}


def dap(t, off, dims):
    a = t[...]
    return bass.AP(tensor=a.tensor, offset=a.offset + off, ap=dims)


def build_kernel(an_scales):
    nc = bacc.Bacc()

    xcatT = nc.dram_tensor("xcatT", [PRE, B], F32R, kind="ExternalInput")
    w0t = nc.dram_tensor("w0t", [HT, PT, 128, 128], F32R, kind="ExternalInput")
    w1t = nc.dram_tensor("w1t", [HT, HT, 128, 128], F32R, kind="ExternalInput")
    w2t = nc.dram_tensor("w2t", [HT, HT, 128, 128], F32R, kind="ExternalInput")
    w3t = nc.dram_tensor("w3t", [HT, HT, 128, 128], F32R, kind="ExternalInput")
    mlpb = nc.dram_tensor("mlpb", [4, HT, 128], F32, kind="ExternalInput")
    wih = nc.dram_tensor("wih", [3, HT, HT, 128, 128], F32R, kind="ExternalInput")
    whh = nc.dram_tensor("whh", [3, HT, HT, 128, 128], F32R, kind="ExternalInput")
    gbias = nc.dram_tensor("gbias", [4, HT, 128], F32, kind="ExternalInput")
    mb = []
    for i in range(3):
        mb.append(dict(
            inproj=nc.dram_tensor(f"inproj{i}", [2 * DIT, DMT, 128, 128], F32R, kind="ExternalInput"),
            convw=nc.dram_tensor(f"convw{i}", [DIT, 128, CK], F32, kind="ExternalInput"),
            convb=nc.dram_tensor(f"convb{i}", [DIT, 128], F32, kind="ExternalInput"),
            xproj=nc.dram_tensor(f"xproj{i}", [DIT, 128, 64], F32R, kind="ExternalInput"),
            dtw=nc.dram_tensor(f"dtw{i}", [DIT, DTR, 128], F32R, kind="ExternalInput"),
            dtb=nc.dram_tensor(f"dtb{i}", [DIT, 128], F32, kind="ExternalInput"),
            Dp=nc.dram_tensor(f"Dp{i}", [DIT, 128], F32, kind="ExternalInput"),
            outproj=nc.dram_tensor(f"outproj{i}", [DIT, DMT, 128, 128], F32R, kind="ExternalInput"),
        ))
    mlp1t = nc.dram_tensor("mlp1t", [MT1, HT, 128, 128], F32R, kind="ExternalInput")
    mlp1b = nc.dram_tensor("mlp1b", [MT1, 128], F32, kind="ExternalInput")

    obsT = nc.dram_tensor("obsT", [MT1, 128, B], F32, kind="ExternalOutput")
    hhatT = nc.dram_tensor("hhatT", [HT, 128, B], F32, kind="ExternalOutput")

    ybuf = nc.dram_tensor("ybuf", [HT, 128, B], F32R)
    xblk = [nc.dram_tensor(f"xblk{i}", [DMT, 128, SEQ, B], F32R) for i in range(3)]
    zbuf = [nc.dram_tensor(f"zbuf{i}", [DIT, 128, SEQ, B], F32) for i in range(3)]
    dtbuf = [nc.dram_tensor(f"dtbuf{i}", [DIT, 128, SEQ, B], F32) for i in range(3)]
    ubuf = [nc.dram_tensor(f"ubuf{i}", [DIT, 128, SEQ, B], F32) for i in range(3)]
    xcbuf = [nc.dram_tensor(f"xcbuf{i}", [DIT, 128, SEQ, B], F32R) for i in range(3)]
    ygbuf = [nc.dram_tensor(f"ygbuf{i}", [DIT, 128, SEQ, B], F32R) for i in range(3)]
    yscan = [nc.dram_tensor(f"yscan{i}", [DIT, 128, B, SEQ], F32) for i in range(3)]
    bcbuf = [nc.dram_tensor(f"bcbuf{i}", [2 * DS, TOK], F32) for i in range(3)]

    with tile.TileContext(nc) as tc:
        # =================== MLP0 ===================
        with tc.tile_pool(name="mlp0w", bufs=4) as wpool, \
             tc.tile_pool(name="mlp0misc", bufs=1) as mpool, \
             tc.tile_pool(name="mlp0ps", bufs=4, space="PSUM") as pspool:
            biases = mpool.tile([128, 4, HT], F32, name="biases")
            nc.sync.dma_start(out=biases, in_=mlpb.rearrange("g t p -> p g t"))
            y_prev = None
            for li, wt_d in enumerate((w0t, w1t, w2t, w3t)):
                kt_n = PT if li == 0 else HT
                y_cur = mpool.tile([128, HT, B], F32R, tag=f"y{li % 2}", name=f"y{li}")
                for mt in range(HT):
                    ps = pspool.tile([128, B], F32, tag="ps", name="ps0")
                    for kt in range(kt_n):
                        wt_s = wpool.tile([128, 128], F32R, tag="w", name="w0s")
                        nc.sync.dma_start(out=wt_s, in_=wt_d[mt, kt, :, :])
                        if li == 0:
                            rhs_t = wpool.tile([128, B], F32R, tag="r", name="r0s")
                            nc.sync.dma_start(out=rhs_t, in_=xcatT[kt * 128:(kt + 1) * 128, :])
                            rhs = rhs_t[:]
                        else:
                            rhs = y_prev[:, kt, :]
                        nc.tensor.matmul(ps[:], wt_s[:], rhs, start=(kt == 0), stop=(kt == kt_n - 1))
                    func = AF.Relu if li < 3 else AF.Identity
                    nc.scalar.activation(y_cur[:, mt, :], ps[:], func, bias=biases[:, li, mt:mt + 1])
                y_prev = y_cur
            for mt in range(HT):
                nc.sync.dma_start(out=ybuf[mt, :, :], in_=y_prev[:, mt, :])

        # =================== MAMBA + GRU ===================
        with tc.tile_pool(name="msb", bufs=1) as sbp, \
             tc.tile_pool(name="mw", bufs=4) as wstr, \
             tc.tile_pool(name="mfm", bufs=1) as fm_p, \
             tc.tile_pool(name="mscan", bufs=1) as scan_p, \
             tc.tile_pool(name="mtmp", bufs=2) as tmp_p, \
             tc.tile_pool(name="mps", bufs=2, space="PSUM") as ps_m:

            def mamba_block(i):
                m = mb[i]
                an = an_scales[i]
                cvec = sbp.tile([128, DIT, CK], F32, tag="convw", name="cvec")
                nc.sync.dma_start(out=cvec, in_=m["convw"].rearrange("t p k -> p t k"))
                cb = sbp.tile([128, DIT], F32, tag="convb", name="cb")
                nc.sync.dma_start(out=cb, in_=m["convb"].rearrange("t p -> p t"))
                dtb_t = sbp.tile([128, DIT], F32, tag="dtb", name="dtb_t")
                nc.sync.dma_start(out=dtb_t, in_=m["dtb"].rearrange("t p -> p t"))
                dp_t = sbp.tile([128, DIT], F32, tag="Dp", name="dp_t")
                nc.sync.dma_start(out=dp_t, in_=m["Dp"].rearrange("t p -> p t"))
                for bh in range(BH):
                    b0 = bh * 128
                    tok_dims = [[SEQ * B, 128], [B, SEQ], [1, 128]]

                    def tok_ap(buf, dt_i):
                        return dap(buf, dt_i * 128 * SEQ * B + b0, tok_dims)

                    # ---- in_proj ----
                    rhs_tiles = []
                    for dmt in range(DMT):
                        rt = tmp_p.tile([128, SEQ, 128], F32R, tag="inrhs", bufs=4, name="inrhs")
                        if i == 0:
                            nc.sync.dma_start(out=rt, in_=dap(
                                ybuf, dmt * 128 * B + b0,
                                [[B, 128], [4 * 128 * B, SEQ], [1, 128]]))
                        else:
                            nc.sync.dma_start(out=rt, in_=tok_ap(xblk[i - 1], dmt))
                        rhs_tiles.append(rt)
                    for mt in range(2 * DIT):
                        ps = ps_m.tile([128, SEQ * 128], F32, tag="big", name="ipps")
                        for kt in range(DMT):
                            wt_s = wstr.tile([128, 128], F32R, tag="ipw", name="ipw")
                            nc.sync.dma_start(out=wt_s, in_=m["inproj"][mt, kt, :, :])
                            rf = rhs_tiles[kt][:, :, :].rearrange("p a b -> p (a b)")
                            for h_ in range(2):
                                nc.tensor.matmul(ps[:, h_ * 512:(h_ + 1) * 512], wt_s[:],
                                                 rf[:, h_ * 512:(h_ + 1) * 512],
                                                 start=(kt == 0), stop=(kt == DMT - 1))
                        if mt < DIT:
                            dt_i = mt
                            xp = fm_p.tile([128, 3 + SEQ, 128], F32, tag="xipad", bufs=2, name="xp")
                            nc.vector.memset(xp[:, 0:3, :], 0.0)
                            nc.scalar.activation(
                                xp[:, 3:3 + SEQ, :].rearrange("p a b -> p (a b)"), ps[:], AF.Copy)
                            c0 = fm_p.tile([128, SEQ * 128], F32, tag="cacc0", name="c0")
                            c1 = fm_p.tile([128, SEQ * 128], F32, tag="cacc1", name="c1")
                            sh = lambda j: xp[:, j:j + SEQ, :].rearrange("p a b -> p (a b)")
                            nc.vector.tensor_scalar_mul(c0[:], sh(0), cvec[:, dt_i, 0:1])
                            nc.vector.scalar_tensor_tensor(c1[:], sh(1), cvec[:, dt_i, 1:2], c0[:], ALU.mult, ALU.add)
                            nc.vector.scalar_tensor_tensor(c0[:], sh(2), cvec[:, dt_i, 2:3], c1[:], ALU.mult, ALU.add)
                            nc.vector.scalar_tensor_tensor(c1[:], sh(3), cvec[:, dt_i, 3:4], c0[:], ALU.mult, ALU.add)
                            xct = fm_p.tile([128, SEQ * 128], F32R, tag="xct", bufs=2, name="xct")
                            nc.scalar.activation(xct[:], c1[:], AF.Silu, bias=cb[:, dt_i:dt_i + 1])
                            nc.sync.dma_start(out=tok_ap(xcbuf[i], dt_i), in_=xct[:])
                        else:
                            zev = fm_p.tile([128, SEQ * 128], F32, tag="zev", bufs=1, name="zev")
                            nc.scalar.activation(zev[:], ps[:], AF.Copy)
                            nc.sync.dma_start(out=tok_ap(zbuf[i], mt - DIT), in_=zev[:])
                    # ---- x_proj ----
                    xdbl_ps = ps_m.tile([64, SEQ * 128], F32, tag="big", name="xpps")
                    for kt in range(DIT):
                        wt_s = wstr.tile([128, 64], F32R, tag="xpw", name="xpw")
                        nc.sync.dma_start(out=wt_s, in_=m["xproj"][kt, :, :])
                        xc_kt = fm_p.tile([128, SEQ * 128], F32R, tag="xcs", bufs=2, name="xcs")
                        nc.sync.dma_start(out=xc_kt, in_=tok_ap(xcbuf[i], kt))
                        for h_ in range(2):
                            nc.tensor.matmul(xdbl_ps[:, h_ * 512:(h_ + 1) * 512], wt_s[:],
                                             xc_kt[:, h_ * 512:(h_ + 1) * 512],
                                             start=(kt == 0), stop=(kt == DIT - 1))
                    xdbl = sbp.tile([64, SEQ * 128], F32R, tag="xdbl", name="xdbl")
                    nc.scalar.activation(xdbl[:], xdbl_ps[:], AF.Copy)
                    # B,C rows -> DRAM for partition-broadcast
                    nc.sync.dma_start(out=bcbuf[i][:, :], in_=xdbl[DTR:DTR + 2 * DS, :].bitcast(F32))
                    # ---- dt, u ----
                    for dt_i in range(DIT):
                        wt_s = wstr.tile([DTR, 128], F32R, tag="dtw", name="dtw")
                        nc.sync.dma_start(out=wt_s, in_=m["dtw"][dt_i, :, :])
                        ps = ps_m.tile([128, SEQ * 128], F32, tag="big", name="dtps")
                        for h_ in range(2):
                            nc.tensor.matmul(ps[:, h_ * 512:(h_ + 1) * 512], wt_s[:],
                                             xdbl[0:DTR, h_ * 512:(h_ + 1) * 512],
                                             start=True, stop=True)
                        dtt = fm_p.tile([128, SEQ * 128], F32, tag="dtt", bufs=2, name="dtt")
                        nc.scalar.activation(dtt[:], ps[:], AF.Exp, bias=dtb_t[:, dt_i:dt_i + 1])
                        nc.scalar.activation(dtt[:], dtt[:], AF.Ln, bias=1.0)
                        nc.sync.dma_start(out=tok_ap(dtbuf[i], dt_i), in_=dtt[:])
                        ut = fm_p.tile([128, SEQ * 128], F32, tag="ut", bufs=2, name="ut")
                        xc_kt = fm_p.tile([128, SEQ * 128], F32R, tag="xcs", bufs=2, name="xcs2")
                        nc.sync.dma_start(out=xc_kt, in_=tok_ap(xcbuf[i], dt_i))
                        nc.vector.tensor_tensor(out=ut[:], in0=dtt[:], in1=xc_kt[:].bitcast(F32), op=ALU.mult)
                        nc.sync.dma_start(out=tok_ap(ubuf[i], dt_i), in_=ut[:])
                    # ---- scan ----
                    for ng in range(NG):
                        Bbc = scan_p.tile([128, NPG, SEQ, 128], F32, tag="Bbc", name="Bbc")
                        Cbc = scan_p.tile([128, NPG, SEQ, 128], F32, tag="Cbc", name="Cbc")
                        for (dst, base) in ((Bbc, 0), (Cbc, DS)):
                            for nr in range(NPG):
                                n_ = ng * NPG + nr
                                nc.sync.dma_start(
                                    out=dst[:, nr, :, :],
                                    in_=dap(bcbuf[i], (base + n_) * TOK,
                                            [[0, 128], [1, TOK]]))
                        for dt_i in range(DIT):
                            dtt = fm_p.tile([128, SEQ * 128], F32, tag="dtt", bufs=2, name="dtt2")
                            nc.sync.dma_start(out=dtt, in_=tok_ap(dtbuf[i], dt_i))
                            ut = fm_p.tile([128, SEQ * 128], F32, tag="ut", bufs=2, name="ut2")
                            nc.sync.dma_start(out=ut, in_=tok_ap(ubuf[i], dt_i))
                            dA = scan_p.tile([128, NPG, 128, SEQ], F32, tag="dA", name="dA")
                            dBu = scan_p.tile([128, NPG, 128, SEQ], F32, tag="dBu", name="dBu")
                            hh = scan_p.tile([128, NPG, 128, SEQ], F32, tag="hh", name="hh")
                            dt_bl = dtt[:].rearrange("p (l b) -> p b l", l=SEQ)
                            for nr in range(NPG):
                                n_ = ng * NPG + nr
                                nc.scalar.activation(dA[:, nr, :, :], dt_bl, AF.Exp,
                                                     scale=float(an[n_]))
                            nc.vector.memset(dA[:, :, :, 0:1], 0.0)
                            u_r = ut[:].rearrange("p (l b) -> p b l", l=SEQ)
                            u_b = bass.AP(tensor=u_r.tensor, offset=u_r.offset,
                                          ap=[u_r.ap[0], [0, NPG], u_r.ap[1], u_r.ap[2]])
                            nc.vector.tensor_tensor(out=dBu[:, :, :, :], in0=u_b, in1=Bbc[:, :, :, :].transpose([0, 1, 3, 2]), op=ALU.mult)
                            flat = lambda t: t[:, :, :, :].rearrange("p a b c -> p (a b c)")
                            nc.vector.tensor_tensor_scan(flat(hh), flat(dA), flat(dBu), 0.0, ALU.mult, ALU.add)
                            nc.vector.tensor_tensor(out=dBu[:, :, :, :], in0=hh[:, :, :, :], in1=Cbc[:, :, :, :].transpose([0, 1, 3, 2]), op=ALU.mult)
                            red = tmp_p.tile([128, 128, SEQ], F32, tag="red", name="red")
                            nc.vector.tensor_reduce(out=red[:, :, :], in_=dBu[:, :, :, :].transpose([0, 2, 3, 1]),
                                                    axis=mybir.AxisListType.X, op=ALU.add)
                            ysc_ap = dap(yscan[i], dt_i * 128 * SEQ * B + b0 * SEQ,
                                         [[SEQ * B, 128], [SEQ, 128], [1, SEQ]])
                            if ng == 0:
                                nc.sync.dma_start(out=ysc_ap, in_=red[:, :, :])
                            else:
                                yacc = tmp_p.tile([128, 128, SEQ], F32, tag="yacc", name="yacc")
                                nc.sync.dma_start(out=yacc, in_=ysc_ap)
                                nc.vector.tensor_tensor(out=red[:, :, :], in0=yacc[:, :, :], in1=red[:, :, :], op=ALU.add)
                                nc.sync.dma_start(out=ysc_ap, in_=red[:, :, :])
                    # ---- gate ----
                    for dt_i in range(DIT):
                        ya = tmp_p.tile([128, 128, SEQ], F32, tag="yacc", name="ya")
                        nc.sync.dma_start(out=ya, in_=dap(
                            yscan[i], dt_i * 128 * SEQ * B + b0 * SEQ,
                            [[SEQ * B, 128], [SEQ, 128], [1, SEQ]]))
                        xct = fm_p.tile([128, SEQ * 128], F32R, tag="xcs", bufs=2, name="xcg")
                        nc.sync.dma_start(out=xct, in_=tok_ap(xcbuf[i], dt_i))
                        zt = tmp_p.tile([128, SEQ * 128], F32, tag="zt", bufs=1, name="zt")
                        nc.sync.dma_start(out=zt, in_=tok_ap(zbuf[i], dt_i))
                        zs = tmp_p.tile([128, SEQ * 128], F32, tag="zs", bufs=1, name="zs")
                        nc.scalar.activation(zs[:], zt[:], AF.Silu)
                        yg = tmp_p.tile([128, SEQ, 128], F32, tag="yg", bufs=1, name="yg")
                        nc.vector.scalar_tensor_tensor(
                            yg[:, :, :], xct[:].rearrange("p (l b) -> p l b", l=SEQ).bitcast(F32),
                            dp_t[:, dt_i:dt_i + 1],
                            ya[:, :, :].transpose([0, 2, 1]), ALU.mult, ALU.add)
                        ygr = tmp_p.tile([128, SEQ * 128], F32R, tag="ygr", bufs=1, name="ygr")
                        nc.vector.tensor_tensor(out=ygr[:].bitcast(F32),
                                                in0=yg[:, :, :].rearrange("p a b -> p (a b)"),
                                                in1=zs[:], op=ALU.mult)
                        nc.sync.dma_start(out=tok_ap(ygbuf[i], dt_i), in_=ygr[:])
                    # ---- out_proj ----
                    for dmt in range(DMT):
                        ps = ps_m.tile([128, SEQ * 128], F32, tag="big", name="opps")
                        for kt in range(DIT):
                            wt_s = wstr.tile([128, 128], F32R, tag="opw", name="opw")
                            nc.sync.dma_start(out=wt_s, in_=m["outproj"][kt, dmt, :, :])
                            ygr = fm_p.tile([128, SEQ * 128], F32R, tag="ygs", bufs=2, name="ygs")
                            nc.sync.dma_start(out=ygr, in_=tok_ap(ygbuf[i], kt))
                            for h_ in range(2):
                                nc.tensor.matmul(ps[:, h_ * 512:(h_ + 1) * 512], wt_s[:],
                                                 ygr[:, h_ * 512:(h_ + 1) * 512],
                                                 start=(kt == 0), stop=(kt == DIT - 1))
                        ot = tmp_p.tile([128, SEQ * 128], F32R, tag="oevac", bufs=1, name="ot")
                        nc.scalar.activation(ot[:], ps[:], AF.Copy)
                        nc.sync.dma_start(out=tok_ap(xblk[i], dmt), in_=ot[:])

            def gru():
                gb = sbp.tile([128, 4, HT], F32, tag="gbias", name="gb")
                nc.sync.dma_start(out=gb, in_=gbias.rearrange("g t p -> p g t"))
                with tc.tile_pool(name="gru_w", bufs=2) as gw, \
                     tc.tile_pool(name="gru_rhs", bufs=2) as gr, \
                     tc.tile_pool(name="gru_t", bufs=1) as gt, \
                     tc.tile_pool(name="gru_ps", bufs=1, space="PSUM") as gps:
                    for mt in range(HT):
                        ps_r = gps.tile([128, B], F32, tag="r", name="psr")
                        ps_z = gps.tile([128, B], F32, tag="z", name="psz")
                        ps_in = gps.tile([128, B], F32, tag="in", name="psin")
                        ps_hn = gps.tile([128, B], F32, tag="hn", name="pshn")
                        for kt in range(HT):
                            y3t = gr.tile([128, B], F32R, tag="y3", name="y3t")
                            nc.sync.dma_start(out=y3t, in_=ybuf[kt, :, :])
                            hTt = gr.tile([128, B], F32R, tag="hT", name="hTt")
                            nc.sync.dma_start(out=hTt, in_=xcatT[OBS + kt * 128:OBS + (kt + 1) * 128, :])
                            first = kt == 0
                            last = kt == HT - 1
                            for g, pst in ((0, ps_r), (1, ps_z), (2, ps_in)):
                                wt_s = gw.tile([128, 128], F32R, tag=f"wi{g}", name="wis")
                                nc.sync.dma_start(out=wt_s, in_=wih[g, mt, kt, :, :])
                                nc.tensor.matmul(pst[:], wt_s[:], y3t[:], start=first,
                                                 stop=(last and g == 2))
                            for g, pst in ((0, ps_r), (1, ps_z), (2, ps_hn)):
                                wt_s = gw.tile([128, 128], F32R, tag=f"wh{g}", name="whs")
                                nc.sync.dma_start(out=wt_s, in_=whh[g, mt, kt, :, :])
                                nc.tensor.matmul(pst[:], wt_s[:], hTt[:], start=(first and g == 2),
                                                 stop=last)
                        r_t = gt.tile([128, B], F32, tag="r", bufs=2, name="rt")
                        nc.scalar.activation(r_t[:], ps_r[:], AF.Sigmoid, bias=gb[:, 0, mt:mt + 1])
                        z_t = gt.tile([128, B], F32, tag="z", bufs=2, name="zt2")
                        nc.scalar.activation(z_t[:], ps_z[:], AF.Sigmoid, bias=gb[:, 1, mt:mt + 1])
                        in_t = gt.tile([128, B], F32, tag="in", bufs=2, name="int")
                        nc.scalar.activation(in_t[:], ps_in[:], AF.Identity, bias=gb[:, 2, mt:mt + 1])
                        hn_t = gt.tile([128, B], F32, tag="hn", bufs=2, name="hnt")
                        nc.scalar.activation(hn_t[:], ps_hn[:], AF.Identity, bias=gb[:, 3, mt:mt + 1])
                        t1 = gt.tile([128, B], F32, tag="t1", bufs=2, name="t1")
                        nc.vector.tensor_tensor(out=t1[:], in0=r_t[:], in1=hn_t[:], op=ALU.mult)
                        nc.vector.tensor_tensor(out=t1[:], in0=t1[:], in1=in_t[:], op=ALU.add)
                        n_t = gt.tile([128, B], F32, tag="n", bufs=2, name="nt")
                        nc.scalar.activation(n_t[:], t1[:], AF.Tanh)
                        hsl = gr.tile([128, B], F32R, tag="hT", name="hsl")
                        nc.sync.dma_start(out=hsl, in_=xcatT[OBS + mt * 128:OBS + (mt + 1) * 128, :])
                        t2 = gt.tile([128, B], F32, tag="t2", bufs=2, name="t2")
                        nc.vector.tensor_tensor(out=t2[:], in0=hsl[:].bitcast(F32), in1=n_t[:], op=ALU.subtract)
                        nc.vector.tensor_tensor(out=t2[:], in0=t2[:], in1=z_t[:], op=ALU.mult)
                        nc.vector.tensor_tensor(out=t2[:], in0=t2[:], in1=n_t[:], op=ALU.add)
                        nc.sync.dma_start(out=hhatT[mt, :, :], in_=t2[:])

            mamba_block(0)
            gru()
            mamba_block(1)
            mamba_block(2)

        # =================== MLP1 ===================
        with tc.tile_pool(name="m1w", bufs=4) as wpool, \
             tc.tile_pool(name="m1o", bufs=4) as opool, \
             tc.tile_pool(name="m1ps", bufs=4, space="PSUM") as pspool:
            b1t = opool.tile([128, MT1], F32, tag="b", bufs=1, name="b1t")
            nc.sync.dma_start(out=b1t, in_=mlp1b.rearrange("t p -> p t"))
            for mt in range(MT1):
                ps = pspool.tile([128, B], F32, tag="ps", name="ps1")
                for kt in range(HT):
                    wt_s = wpool.tile([128, 128], F32R, tag="w", name="w1s")
                    nc.sync.dma_start(out=wt_s, in_=mlp1t[mt, kt, :, :])
                    rhs_t = wpool.tile([128, B], F32R, tag="r", name="r1s")
                    nc.sync.dma_start(out=rhs_t, in_=xblk[2][kt % 4, :, kt // 4, :])
                    nc.tensor.matmul(ps[:], wt_s[:], rhs_t[:], start=(kt == 0), stop=(kt == HT - 1))
                ot = opool.tile([128, B], F32, tag="o", name="o1t")
                nc.scalar.activation(ot[:], ps[:], AF.Identity, bias=b1t[:, mt:mt + 1])
                nc.sync.dma_start(out=obsT[mt, :, :], in_=ot[:])

    nc.compile()
    return nc


def _tiles(WT, ktiles, mtiles):
    return np.ascontiguousarray(
        WT.reshape(ktiles, 128, mtiles, 128).transpose(2, 0, 1, 3))


def kernel(obs_latent, h_state, action, params):
    obs_latent = np.asarray(obs_latent, np.float32)
    h_state = np.asarray(h_state, np.float32)
    action = np.asarray(action, np.float32)

    mlp0 = params["mlp0"]
    gru = params["gru"]
    mambas = params["mamba"]
    W1, b1o = params["mlp1"]

    an_scales = []
    for mp in mambas:
        A = -np.exp(np.asarray(mp["A_log"], np.float32))
        assert np.allclose(A, A[0:1, :], atol=1e-5), "A must be d-independent"
        an_scales.append([float(x) for x in A[0]])

    key = tuple(tuple(a) for a in an_scales)
    if key not in _BUILD_CACHE:
        _BUILD_CACHE[key] = build_kernel(an_scales)
    nc = _BUILD_CACHE[key]

    xcat = np.concatenate([obs_latent, h_state, action], axis=1)

    shared = {}
    for li, (W, b) in enumerate(mlp0):
        W = np.asarray(W, np.float32)
        shared[f"w{li}t"] = _tiles(np.ascontiguousarray(W.T), W.shape[1] // 128, W.shape[0] // 128)
    shared["mlpb"] = np.stack([np.asarray(b, np.float32).reshape(HT, 128) for _, b in mlp0])
    wih_ = np.asarray(gru["w_ih"], np.float32)
    whh_ = np.asarray(gru["w_hh"], np.float32)
    shared["wih"] = np.stack([_tiles(np.ascontiguousarray(wih_[g * HID:(g + 1) * HID, :].T), HT, HT) for g in range(3)])
    shared["whh"] = np.stack([_tiles(np.ascontiguousarray(whh_[g * HID:(g + 1) * HID, :].T), HT, HT) for g in range(3)])
    bih = np.asarray(gru["b_ih"], np.float32)
    bhh = np.asarray(gru["b_hh"], np.float32)
    shared["gbias"] = np.stack([
        (bih[0:HID] + bhh[0:HID]).reshape(HT, 128),
        (bih[HID:2 * HID] + bhh[HID:2 * HID]).reshape(HT, 128),
        bih[2 * HID:].reshape(HT, 128),
        bhh[2 * HID:].reshape(HT, 128)])
    for i, mp in enumerate(mambas):
        ip = np.asarray(mp["in_proj"], np.float32)
        shared[f"inproj{i}"] = _tiles(np.ascontiguousarray(ip.T), DMT, 2 * DIT)
        shared[f"convw{i}"] = np.asarray(mp["conv_w"], np.float32).reshape(DIT, 128, CK)
        shared[f"convb{i}"] = np.asarray(mp["conv_b"], np.float32).reshape(DIT, 128)
        xp_ = np.asarray(mp["x_proj"], np.float32)
        shared[f"xproj{i}"] = np.ascontiguousarray(xp_.T.reshape(DIT, 128, 64))
        dw = np.asarray(mp["dt_w"], np.float32)
        shared[f"dtw{i}"] = np.ascontiguousarray(dw.T.reshape(DTR, DIT, 128).transpose(1, 0, 2))
        shared[f"dtb{i}"] = np.asarray(mp["dt_b"], np.float32).reshape(DIT, 128)
        shared[f"Dp{i}"] = np.asarray(mp["D"], np.float32).reshape(DIT, 128)
        op_ = np.asarray(mp["out_proj"], np.float32)
        shared[f"outproj{i}"] = _tiles(np.ascontiguousarray(op_.T), DIT, DMT)
    W1 = np.asarray(W1, np.float32)
    shared["mlp1t"] = _tiles(np.ascontiguousarray(W1.T), HT, MT1)
    shared["mlp1b"] = np.asarray(b1o, np.float32).reshape(MT1, 128)

    in_maps = []
    for c in range(NCORES):
        im = dict(shared)
        im["xcatT"] = np.ascontiguousarray(xcat[c * B:(c + 1) * B].T)
        in_maps.append(im)

    import os as _os
    _trace = _os.environ.get("KM_TRACE") == "1"
    res = run_bass_kernel_spmd(nc, in_maps, core_ids=list(range(NCORES)), trace=_trace)
    if _trace:
        print("TRACE exec_time_ns:", res.exec_time_ns, "mean:", res.mean_exec_time_ns)

    obs_out = np.empty((NCORES * B, OBS), np.float32)
    h_out = np.empty((NCORES * B, HID), np.float32)
    for c in range(NCORES):
        obs_out[c * B:(c + 1) * B] = res.results[c]["obsT"].reshape(OBS, B).T
        h_out[c * B:(c + 1) * B] = res.results[c]["hhatT"].reshape(HID, B).T
    return obs_out, h_out
